# revision 8
# baseline (speedup 1.0000x reference)
"""Trainium2 Bass kernel for DirectedGraphConv.

Reference math (per batch b, node n):
    out = feature + einsum("bni,doi->bno", feature, weights) + bias[graph].sum(axis=2)

Identities used:
  * einsum sums over BOTH directions d and input dim i, so it equals
    F @ (W0 + W1)^T.  The "+ feature" residual folds in as +Identity on the
    transposed weight (added in bf16 on the diagonal blocks).
  * bias[graph].sum(axis=2) only depends on the per-row label histogram:
        CountT[l, bn] = #{m : graph[bn, m] == l}   (16 labels)
        bias_term     = CountT^T @ bias             ([BN,16] @ [16,512])

Sharding: data-parallel over batch; 32 batches -> 4 per NeuronCore x 8 cores.
weights/bias replicated.  Each core runs an identical program (SPMD).

Performance design (v2):
  * Two HW DGE queues in parallel: the sync queue carries W (first - it
    gates the most downstream work) then F per-batch; the scalar queue
    carries graph+bias in and ALL outputs out, so outputs start the moment
    each batch is ready instead of queueing behind input transfers.
  * graph is narrowed to int8 on the host (values 0..15, lossless): 64KB
    per core instead of 512KB of int64-as-int32-pairs.
  * Matmul operands are bf16 (single-pass PE streaming, single-pass
    transposes).  W'^T gets +1 on the diagonal in bf16; counts are exact
    integers in bf16; PSUM accumulates f32.  Rel err ~4e-3 << 2e-2 budget.
  * Histogram: graph -> bf16 -> 4 PE transposes -> is_equal per label
    (split DVE/gpsimd) -> one-hot selector count matmuls accumulating
    CountT[16, bn] in one PSUM bank (PE).
  * Per-oc W pipeline (DMA -> Wsum add (gpsimd, bf16 out) -> 4 bf16 PE
    transposes -> DVE/ACT copy -> gpsimd diag +1) and per-batch F pipeline
    (DMA -> 4 f32 PE transposes -> bf16 copy -> matmuls -> copy -> out DMA)
    keep the tail after the last input byte short.
  * A burst of dependency-free warm-up matmuls runs during the input DMA
    window so the PE HAM clock reaches 2.4 GHz before real matmuls start.
"""

import numpy as np

B, N, D = 32, 128, 512
DIR = 2
L = 16  # num labels
NC = 8  # neuron cores
BPC = B // NC  # batches per core = 4
BN = BPC * N  # rows per core = 512
P = 128
KC = D // P  # 4 k-chunks
WARMUP_MMS = 30

_prog_cache: dict = {}


def _build():
    import concourse.bass as bass  # noqa: F401
    import concourse.mybir as mybir
    import concourse.tile as tile
    from concourse import bacc
    from concourse.masks import make_identity

    f32 = mybir.dt.float32
    bf16 = mybir.dt.bfloat16
    i8 = mybir.dt.int8

    nc = bacc.Bacc(
        "TRN2",
        target_bir_lowering=False,
        debug=False,
        num_devices=NC,
    )

    feat = nc.dram_tensor("feature", [BPC, N, D], f32, kind="ExternalInput").ap()
    graph = nc.dram_tensor("graph", [BPC, N, N], i8, kind="ExternalInput").ap()
    wts = nc.dram_tensor("weights", [DIR, D, D], f32, kind="ExternalInput").ap()
    bias = nc.dram_tensor("bias", [L, D], f32, kind="ExternalInput").ap()
    out = nc.dram_tensor("out", [BPC, N, D], f32, kind="ExternalOutput").ap()

    with tile.TileContext(nc) as tc:
        with (
            tc.tile_pool(name="const", bufs=1) as cpool,
            tc.tile_pool(name="work", bufs=1) as wpool,
            tc.tile_pool(name="psum", bufs=1, space="PSUM") as ppool,
        ):
            # ---- constants built on-chip (gpsimd) ----
            ident_bf = cpool.tile([P, P], bf16)
            make_identity(nc, ident_bf)
            ident = cpool.tile([P, P], f32)
            make_identity(nc, ident)
            # esel[m, 16*l + j] = 1.0 iff j == l  (label-selector stationaries)
            esel = cpool.tile([P, L * L], bf16)
            nc.gpsimd.memset(esel, 0.0)
            esel3 = esel.rearrange("p (l j) -> p l j", l=L)
            nc.gpsimd.affine_select(
                out=esel3,
                in_=esel3,
                compare_op=mybir.AluOpType.not_equal,
                fill=1.0,
                base=0,
                pattern=[[1, L], [-1, L]],
                channel_multiplier=0,
            )

            # ---- ACT activation-table preload (first Copy loads the table) ----
            act_warm = cpool.tile([P, 2], f32)
            nc.scalar.copy(out=act_warm[:, 0:1], in_=ident[:, 0:1])

            # ---- HAM warm-up: dependency-free matmuls during the DMA wait ----
            psum_warm = ppool.tile([P, P], f32, tag="warm", bufs=1)
            for _ in range(WARMUP_MMS):
                nc.tensor.matmul(
                    out=psum_warm,
                    lhsT=ident_bf,
                    rhs=ident_bf,
                    start=True,
                    stop=True,
                )

            # ---- DMA inputs ----
            # scalar ring: graph (int8), bias (and, later, all outputs).
            # sync ring: W per-oc then F per-batch.
            G_sb = wpool.tile([P, BPC, N], i8)
            nc.scalar.dma_start(out=G_sb, in_=graph.rearrange("b n m -> n b m"))

            bias_sb = wpool.tile([L, D], f32)
            nc.scalar.dma_start(out=bias_sb, in_=bias)

            W_sb = wpool.tile([P, KC, DIR, D], f32)
            for oc in range(KC):
                nc.sync.dma_start(
                    out=W_sb[:, oc, :, :],
                    in_=wts[:, oc * P : (oc + 1) * P, :].rearrange("d p i -> p d i"),
                )

            F_sb = wpool.tile([P, BPC, D], f32)
            for b in range(BPC):
                nc.sync.dma_start(out=F_sb[:, b, :], in_=feat[b])

            # bias -> bf16 (gpsimd; small)
            bias_bf = wpool.tile([L, D], bf16)
            nc.gpsimd.tensor_copy(out=bias_bf, in_=bias_sb)

            # ---- W path: Wsum (bf16, gpsimd) per oc -> 4 bf16 PE transposes
            #      -> DVE/ACT copy -> gpsimd diag +1 (the +feature residual) --
            Wsum = wpool.tile([P, KC, D], bf16)
            for oc in range(KC):
                nc.gpsimd.tensor_tensor(
                    out=Wsum[:, oc, :],
                    in0=W_sb[:, oc, 0, :],
                    in1=W_sb[:, oc, 1, :],
                    op=mybir.AluOpType.add,
                )

            WT = wpool.tile([P, KC, D], bf16)  # [i', c, o]
            for oc in range(KC):
                psum_wt = ppool.tile([P, KC, P], bf16, tag="wt", bufs=2)
                for c in range(KC):
                    nc.tensor.matmul(
                        out=psum_wt[:, c, :],
                        lhsT=Wsum[:, oc, c * P : (c + 1) * P],
                        rhs=ident_bf,
                        is_transpose=True,
                        start=True,
                        stop=True,
                    )
                if oc % 2 == 0:
                    nc.vector.tensor_copy(
                        out=WT[:, :, oc * P : (oc + 1) * P], in_=psum_wt
                    )
                else:
                    nc.scalar.copy(
                        out=WT[:, :, oc * P : (oc + 1) * P], in_=psum_wt
                    )
                # +Identity on the diagonal block (c == oc)
                nc.gpsimd.tensor_tensor(
                    out=WT[:, oc, oc * P : (oc + 1) * P],
                    in0=WT[:, oc, oc * P : (oc + 1) * P],
                    in1=ident_bf,
                    op=mybir.AluOpType.add,
                )

            # ---- graph: int8 -> bf16 cast, then 4 PE transposes ----
            gbf = wpool.tile([P, BPC, N], bf16)
            nc.vector.tensor_copy(out=gbf, in_=G_sb)

            psum_gt = ppool.tile([P, BN], bf16, tag="ft", bufs=2)
            for c in range(BPC):
                nc.tensor.matmul(
                    out=psum_gt[:, c * P : (c + 1) * P],
                    lhsT=gbf[:, c, :],
                    rhs=ident_bf,
                    is_transpose=True,
                    start=True,
                    stop=True,
                )
            gT = wpool.tile([P, BN], bf16)  # [m, bn]
            nc.vector.tensor_copy(out=gT, in_=psum_gt)

            # ---- histogram: EQ (DVE/gpsimd) + selector matmuls -> CountT ----
            EQ = wpool.tile([P, 4, BN], bf16)
            psum_cnt = ppool.tile([L, BN], f32, tag="cnt", bufs=1)
            for l in range(L):
                eng = nc.vector if l % 2 == 0 else nc.gpsimd
                eng.tensor_scalar(
                    out=EQ[:, l % 4, :],
                    in0=gT,
                    scalar1=float(l),
                    scalar2=None,
                    op0=mybir.AluOpType.is_equal,
                )
                nc.tensor.matmul(
                    out=psum_cnt,
                    lhsT=esel[:, l * L : (l + 1) * L],
                    rhs=EQ[:, l % 4, :],
                    start=(l == 0),
                    stop=(l == L - 1),
                )
            cntT = wpool.tile([L, BN], bf16)  # counts <= 128: exact in bf16
            nc.scalar.copy(out=cntT, in_=psum_cnt)

            # ---- F path: per-batch f32 PE transposes, bf16 copies ----
            FT = wpool.tile([P, KC, BN], bf16)  # [i', c, bn]
            for b in range(BPC):
                psum_ft = ppool.tile([P, KC, P], f32, tag="ft", bufs=2)
                for c in range(KC):
                    nc.tensor.matmul(
                        out=psum_ft[:, c, :],
                        lhsT=F_sb[:, b, c * P : (c + 1) * P],
                        rhs=ident,
                        is_transpose=True,
                        start=True,
                        stop=True,
                    )
                if b % 2 == 0:
                    nc.vector.tensor_copy(
                        out=FT[:, :, b * P : (b + 1) * P], in_=psum_ft
                    )
                else:
                    nc.scalar.copy(out=FT[:, :, b * P : (b + 1) * P], in_=psum_ft)

            # ---- main matmuls (bf16) + bias term, copy out, store ----
            out_sb = wpool.tile([P, BPC, D], f32)
            for b in range(BPC):
                psum_out = ppool.tile([P, D], f32, tag="out", bufs=2)
                for c in range(KC):
                    nc.tensor.matmul(
                        out=psum_out,
                        lhsT=FT[:, c, b * P : (b + 1) * P],
                        rhs=WT[:, c, :],
                        start=(c == 0),
                        stop=False,
                    )
                nc.tensor.matmul(
                    out=psum_out,
                    lhsT=cntT[:, b * P : (b + 1) * P],
                    rhs=bias_bf,
                    start=False,
                    stop=True,
                )
                if b == BPC - 1:
                    # last batch: split across DVE+ACT and DMA per half so
                    # the exposed tail chain is shorter
                    h = D // 2
                    nc.vector.tensor_copy(
                        out=out_sb[:, b, 0:h], in_=psum_out[:, 0:h]
                    )
                    nc.scalar.copy(out=out_sb[:, b, h:D], in_=psum_out[:, h:D])
                    nc.scalar.dma_start(out=out[b, :, 0:h], in_=out_sb[:, b, 0:h])
                    nc.scalar.dma_start(out=out[b, :, h:D], in_=out_sb[:, b, h:D])
                else:
                    if b % 2 == 0:
                        nc.vector.tensor_copy(out=out_sb[:, b, :], in_=psum_out)
                    else:
                        nc.scalar.copy(out=out_sb[:, b, :], in_=psum_out)
                    nc.scalar.dma_start(out=out[b], in_=out_sb[:, b, :])

    nc.compile()
    return nc


def _get_prog():
    if "v2" not in _prog_cache:
        _prog_cache["v2"] = _build()
    return _prog_cache["v2"]


def _shard_inputs(feature, graph, weights, bias):
    feature = np.ascontiguousarray(np.asarray(feature), dtype=np.float32)
    weights = np.ascontiguousarray(np.asarray(weights), dtype=np.float32)
    bias = np.ascontiguousarray(np.asarray(bias), dtype=np.float32)
    # labels are 0..15: int8 is a lossless narrowing
    g8 = np.ascontiguousarray(np.asarray(graph).astype(np.int8))
    in_maps = []
    for core in range(NC):
        sl = slice(core * BPC, (core + 1) * BPC)
        in_maps.append(
            {
                "feature": np.ascontiguousarray(feature[sl]),
                "graph": np.ascontiguousarray(g8[sl]),
                "weights": weights,
                "bias": bias,
            }
        )
    return in_maps


def _run(feature, graph, weights, bias, trace=False):
    from concourse.bass_utils import run_bass_kernel_spmd

    in_maps = _shard_inputs(feature, graph, weights, bias)
    nc = _get_prog()
    res = run_bass_kernel_spmd(nc, in_maps, core_ids=list(range(NC)), trace=trace)
    out = np.concatenate([r["out"] for r in res.results], axis=0)
    return out, res


def kernel(feature, graph, weights, bias):
    out, _ = _run(feature, graph, weights, bias, trace=False)
    return out


# revision 16
# speedup vs baseline: 2.6595x; 2.6595x over previous
"""Trainium2 Bass kernel for DirectedGraphConv.

Reference math (per batch b, node n):
    out = feature + einsum("bni,doi->bno", feature, weights) + bias[graph].sum(axis=2)

Identities used:
  * einsum sums over BOTH directions d and input dim i, so it equals
    F @ (W0 + W1)^T.  The "+ feature" residual folds in as +Identity on the
    transposed weight (added in bf16 on the diagonal blocks).
  * bias[graph].sum(axis=2) only depends on the per-row label histogram:
        CountT[l, bn] = #{m : graph[bn, m] == l}   (16 labels)
        bias_term     = CountT^T @ bias             ([BN,16] @ [16,512])

Sharding: data-parallel over batch; 32 batches -> 4 per NeuronCore x 8 cores.
weights/bias replicated.  Each core runs an identical program (SPMD).

Performance design (v2):
  * Two HW DGE queues in parallel: the sync queue carries W (first - it
    gates the most downstream work) then F per-batch; the scalar queue
    carries graph+bias in and ALL outputs out, so outputs start the moment
    each batch is ready instead of queueing behind input transfers.
  * graph is narrowed to int8 on the host (values 0..15, lossless): 64KB
    per core instead of 512KB of int64-as-int32-pairs.
  * Matmul operands are bf16 (single-pass PE streaming, single-pass
    transposes).  W'^T gets +1 on the diagonal in bf16; counts are exact
    integers in bf16; PSUM accumulates f32.  Rel err ~4e-3 << 2e-2 budget.
  * Histogram: graph -> bf16 -> 4 PE transposes -> is_equal per label
    (split DVE/gpsimd) -> one-hot selector count matmuls accumulating
    CountT[16, bn] in one PSUM bank (PE).
  * Per-oc W pipeline (DMA -> Wsum add (gpsimd, bf16 out) -> 4 bf16 PE
    transposes -> DVE/ACT copy -> gpsimd diag +1) and per-batch F pipeline
    (DMA -> 4 f32 PE transposes -> bf16 copy -> matmuls -> copy -> out DMA)
    keep the tail after the last input byte short.
  * A burst of dependency-free warm-up matmuls runs during the input DMA
    window so the PE HAM clock reaches 2.4 GHz before real matmuls start.
"""

import numpy as np

B, N, D = 32, 128, 512
DIR = 2
L = 16  # num labels
NC = 8  # neuron cores
BPC = B // NC  # batches per core = 4
BN = BPC * N  # rows per core = 512
P = 128
KC = D // P  # 4 k-chunks
WARMUP_MMS = 24

_prog_cache: dict = {}


def _build():
    import concourse.bass as bass  # noqa: F401
    import concourse.mybir as mybir
    import concourse.tile as tile
    from concourse import bacc
    from concourse.masks import make_identity

    f32 = mybir.dt.float32
    bf16 = mybir.dt.bfloat16
    i8 = mybir.dt.int8

    nc = bacc.Bacc(
        "TRN2",
        target_bir_lowering=False,
        debug=False,
        num_devices=NC,
    )

    feat = nc.dram_tensor("feature", [BPC, N, D], f32, kind="ExternalInput").ap()
    # graph arrives host-transposed to [n, b, m] so the DMA is contiguous
    graph = nc.dram_tensor("graph", [N, BPC, N], i8, kind="ExternalInput").ap()
    wts = nc.dram_tensor("weights", [DIR, D, D], f32, kind="ExternalInput").ap()
    bias = nc.dram_tensor("bias", [L, D], f32, kind="ExternalInput").ap()
    out = nc.dram_tensor("out", [BPC, N, D], f32, kind="ExternalOutput").ap()

    with tile.TileContext(nc) as tc:
        with (
            tc.tile_pool(name="const", bufs=1) as cpool,
            tc.tile_pool(name="work", bufs=1) as wpool,
            tc.tile_pool(name="psum", bufs=1, space="PSUM") as ppool,
        ):
            # ---- constants built on-chip (gpsimd) ----
            ident_bf = cpool.tile([P, P], bf16)
            make_identity(nc, ident_bf)
            ident = cpool.tile([P, P], f32)
            make_identity(nc, ident)
            # esel[m, 16*l + j] = 1.0 iff j == l  (label-selector stationaries)
            esel = cpool.tile([P, L * L], bf16)
            nc.gpsimd.memset(esel, 0.0)
            esel3 = esel.rearrange("p (l j) -> p l j", l=L)
            nc.gpsimd.affine_select(
                out=esel3,
                in_=esel3,
                compare_op=mybir.AluOpType.not_equal,
                fill=1.0,
                base=0,
                pattern=[[1, L], [-1, L]],
                channel_multiplier=0,
            )

            # ---- ACT activation-table preload (first Copy loads the table) ----
            act_warm = cpool.tile([P, 2], f32)
            nc.scalar.copy(out=act_warm[:, 0:1], in_=ident[:, 0:1])

            # ---- HAM warm-up: dependency-free matmuls during the DMA wait ----
            psum_warm = ppool.tile([P, P], f32, tag="warm", bufs=1)
            for _ in range(WARMUP_MMS):
                nc.tensor.matmul(
                    out=psum_warm,
                    lhsT=ident_bf,
                    rhs=ident_bf,
                    start=True,
                    stop=True,
                )

            # ---- DMA inputs ----
            # scalar ring: graph (int8), bias (and, later, all outputs).
            # sync ring: W per-oc then F per-batch.
            G_sb = wpool.tile([P, BPC, N], i8)
            nc.scalar.dma_start(out=G_sb, in_=graph)

            bias_sb = wpool.tile([L, D], f32)
            nc.scalar.dma_start(out=bias_sb, in_=bias)

            W_sb = wpool.tile([P, KC, DIR, D], f32)
            for oc in range(KC):
                nc.sync.dma_start(
                    out=W_sb[:, oc, :, :],
                    in_=wts[:, oc * P : (oc + 1) * P, :].rearrange("d p i -> p d i"),
                )

            F_sb = wpool.tile([P, BPC, D], f32)
            for b in range(BPC):
                nc.sync.dma_start(out=F_sb[:, b, :], in_=feat[b])

            # bias -> bf16 (gpsimd; small)
            bias_bf = wpool.tile([L, D], bf16)
            nc.gpsimd.tensor_copy(out=bias_bf, in_=bias_sb)

            # ---- W path: Wsum (bf16, gpsimd) per oc -> 4 bf16 PE transposes
            #      -> DVE/ACT copy -> gpsimd diag +1 (the +feature residual) --
            Wsum = wpool.tile([P, KC, D], bf16)
            for oc in range(KC):
                nc.gpsimd.tensor_tensor(
                    out=Wsum[:, oc, :],
                    in0=W_sb[:, oc, 0, :],
                    in1=W_sb[:, oc, 1, :],
                    op=mybir.AluOpType.add,
                )

            WT = wpool.tile([P, KC, D], bf16)  # [i', c, o]
            for oc in range(KC):
                psum_wt = ppool.tile([P, KC, P], bf16, tag="wt", bufs=2)
                for c in range(KC):
                    nc.tensor.matmul(
                        out=psum_wt[:, c, :],
                        lhsT=Wsum[:, oc, c * P : (c + 1) * P],
                        rhs=ident_bf,
                        is_transpose=True,
                        start=True,
                        stop=True,
                    )
                nc.scalar.copy(out=WT[:, :, oc * P : (oc + 1) * P], in_=psum_wt)
                # +Identity on the diagonal block (c == oc); DVE, it's tiny
                nc.vector.tensor_tensor(
                    out=WT[:, oc, oc * P : (oc + 1) * P],
                    in0=WT[:, oc, oc * P : (oc + 1) * P],
                    in1=ident_bf,
                    op=mybir.AluOpType.add,
                )

            # ---- graph: int8 -> bf16 cast, then 4 PE transposes ----
            gbf = wpool.tile([P, BPC, N], bf16)
            nc.vector.tensor_copy(out=gbf, in_=G_sb)

            psum_gt = ppool.tile([P, BN], bf16, tag="ft", bufs=2)
            for c in range(BPC):
                nc.tensor.matmul(
                    out=psum_gt[:, c * P : (c + 1) * P],
                    lhsT=gbf[:, c, :],
                    rhs=ident_bf,
                    is_transpose=True,
                    start=True,
                    stop=True,
                )
            gT = wpool.tile([P, BN], bf16)  # [m, bn]
            nc.vector.tensor_copy(out=gT, in_=psum_gt)

            # ---- histogram: EQ (DVE/gpsimd) + selector matmuls -> CountT ----
            EQ = wpool.tile([P, 4, BN], bf16)
            psum_cnt = ppool.tile([L, BN], f32, tag="cnt", bufs=1)
            for l in range(L):
                nc.vector.tensor_scalar(
                    out=EQ[:, l % 4, :],
                    in0=gT,
                    scalar1=float(l),
                    scalar2=None,
                    op0=mybir.AluOpType.is_equal,
                )
                nc.tensor.matmul(
                    out=psum_cnt,
                    lhsT=esel[:, l * L : (l + 1) * L],
                    rhs=EQ[:, l % 4, :],
                    start=(l == 0),
                    stop=(l == L - 1),
                )
            cntT = wpool.tile([L, BN], bf16)  # counts <= 128: exact in bf16
            nc.vector.tensor_copy(out=cntT, in_=psum_cnt)

            # ---- F path: per-batch f32 PE transposes, bf16 copies ----
            FT = wpool.tile([P, KC, BN], bf16)  # [i', c, bn]
            for b in range(BPC):
                psum_ft = ppool.tile([P, KC, P], f32, tag="ft", bufs=2)
                for c in range(KC):
                    nc.tensor.matmul(
                        out=psum_ft[:, c, :],
                        lhsT=F_sb[:, b, c * P : (c + 1) * P],
                        rhs=ident,
                        is_transpose=True,
                        start=True,
                        stop=True,
                    )
                nc.scalar.copy(out=FT[:, :, b * P : (b + 1) * P], in_=psum_ft)

            # ---- main matmuls (bf16) + bias term, copy out, store ----
            out_sb = wpool.tile([P, BPC, D], f32)
            for b in range(BPC):
                psum_out = ppool.tile([P, D], f32, tag="out", bufs=2)
                for c in range(KC):
                    nc.tensor.matmul(
                        out=psum_out,
                        lhsT=FT[:, c, b * P : (b + 1) * P],
                        rhs=WT[:, c, :],
                        start=(c == 0),
                        stop=False,
                    )
                nc.tensor.matmul(
                    out=psum_out,
                    lhsT=cntT[:, b * P : (b + 1) * P],
                    rhs=bias_bf,
                    start=False,
                    stop=True,
                )
                if b == BPC - 1:
                    # last batch: split across DVE+ACT and DMA per half so
                    # the exposed tail chain is shorter
                    h = D // 2
                    nc.vector.tensor_copy(
                        out=out_sb[:, b, 0:h], in_=psum_out[:, 0:h]
                    )
                    nc.scalar.copy(out=out_sb[:, b, h:D], in_=psum_out[:, h:D])
                    nc.scalar.dma_start(out=out[b, :, 0:h], in_=out_sb[:, b, 0:h])
                    nc.scalar.dma_start(out=out[b, :, h:D], in_=out_sb[:, b, h:D])
                else:
                    if b % 2 == 0:
                        nc.vector.tensor_copy(out=out_sb[:, b, :], in_=psum_out)
                    else:
                        nc.scalar.copy(out=out_sb[:, b, :], in_=psum_out)
                    nc.scalar.dma_start(out=out[b], in_=out_sb[:, b, :])

    nc.compile()
    return nc


def _get_prog():
    if "v2" not in _prog_cache:
        _prog_cache["v2"] = _build()
    return _prog_cache["v2"]


def _shard_inputs(feature, graph, weights, bias):
    feature = np.ascontiguousarray(np.asarray(feature), dtype=np.float32)
    weights = np.ascontiguousarray(np.asarray(weights), dtype=np.float32)
    bias = np.ascontiguousarray(np.asarray(bias), dtype=np.float32)
    # labels are 0..15: int8 is a lossless narrowing; pre-swap to [n, b, m]
    # so the device DMA is fully contiguous
    g8 = np.asarray(graph).astype(np.int8)
    in_maps = []
    for core in range(NC):
        sl = slice(core * BPC, (core + 1) * BPC)
        in_maps.append(
            {
                "feature": np.ascontiguousarray(feature[sl]),
                "graph": np.ascontiguousarray(g8[sl].transpose(1, 0, 2)),
                "weights": weights,
                "bias": bias,
            }
        )
    return in_maps


def _run(feature, graph, weights, bias, trace=False):
    from concourse.bass_utils import run_bass_kernel_spmd

    in_maps = _shard_inputs(feature, graph, weights, bias)
    nc = _get_prog()
    res = run_bass_kernel_spmd(nc, in_maps, core_ids=list(range(NC)), trace=trace)
    out = np.concatenate([r["out"] for r in res.results], axis=0)
    return out, res


def kernel(feature, graph, weights, bias):
    out, _ = _run(feature, graph, weights, bias, trace=False)
    return out
